# revision 4
# baseline (speedup 1.0000x reference)
"""DenseDilatedKnnGraph kernel for 8 Trainium2 NeuronCores.

Input : x (2, 64, 8192, 1) float32
Output: edge_index (2, 2, 8192, 9) int32
  out[0] = nn_idx[..., ::2] of top-18 nearest (L2, channel-normalized points)
  out[1] = center indices (arange broadcast)

Sharding: data-parallel over (batch, query-block): core c handles batch c//4,
queries [(c%4)*2048, (c%4+1)*2048). Each core holds all 8192 candidates.

Per-core pipeline (all ranking done on similarity = <q_raw, c_normalized>;
query-side normalization is a per-row scale and hence order-neutral):
  - normalize candidates (Square -> matmul-reduce -> Rsqrt -> mult)
  - split candidates and raw queries into bf16 hi/lo pairs; the distance
    matmul is hi.hi + (hi.lo + lo.hi) via two accumulating bf16 matmuls
    (~1e-5 abs accuracy, 4x faster than f32 matmul, p-state tolerant)
  - Act copies PSUM chunks to SBUF f32 (negd, double buffered)
  - DVE: 8x max8 over 1024-wide groups -> compact top-8-per-group list (64)
    -> 3 rounds max8/match_replace -> ranks 0..23 -> strided even ranks
    2,4,...,16 -> one full-row max_index for original indices
Rank 0 of the top-18 is always the query itself, filled host-side.
"""

import os
import sys
import time

import numpy as np

try:
    import concourse.bass as bass  # noqa: F401
except ImportError:  # fresh grading dir: make repo importable
    sys.path.append("/opt/trn_rl_repo")

import concourse.bacc as bacc
import concourse.mybir as mybir
import concourse.tile as tile
from concourse.bass_utils import run_bass_kernel_spmd

F32 = mybir.dt.float32
BF16 = mybir.dt.bfloat16
U32 = mybir.dt.uint32
AF = mybir.ActivationFunctionType
ALU = mybir.AluOpType

B = 2          # batch
C = 64         # channels
N = 8192       # points (candidates per core)
Q = 2048       # queries per core
QTS = 128      # queries per tile
GRP = 1024     # first-level group size
NG = N // GRP  # 8 groups
CK = NG * 8    # compact entries per row (64)
NEG_INF = -3.0e38


def build_program(loop_iters: int = 1):
    nc = bacc.Bacc()
    xb_d = nc.dram_tensor("xb", [C, N], F32, kind="ExternalInput")
    xq_d = nc.dram_tensor("xq", [C, Q], F32, kind="ExternalInput")
    out_d = nc.dram_tensor("out", [Q, 8], U32, kind="ExternalOutput")

    with tile.TileContext(nc) as tc:
        with (
            tc.tile_pool(name="const", bufs=1) as cst,
            tc.tile_pool(name="big", bufs=1) as big,
        ):
            ones64 = cst.tile([C, 1], F32)
            nc.gpsimd.memset(ones64[:], 1.0)

            # persistent bf16 operands
            sq_c = big.tile([128, N], BF16)   # candidates: [hi; lo]
            rx_c = big.tile([128, N], BF16)   # candidates: [lo; hi]
            sq_q = big.tile([128, Q], BF16)   # queries:    [hi; lo]

            with (
                tc.tile_pool(name="nsb", bufs=2) as nsb,
                tc.tile_pool(name="nps", bufs=2, space="PSUM") as nps,
            ):
                # ---- candidates: load, normalize, bf16 split ----
                xc = nsb.tile([C, N], F32, tag="xc", bufs=1)
                nc.sync.dma_start(xc[:], xb_d[:])
                xn = nsb.tile([C, N], F32, tag="xn", bufs=1)
                rinv = nsb.tile([1, N], F32, tag="rinv", bufs=1)
                nb = nsb.tile([C, N], F32, tag="nb", bufs=1)
                for c0 in range(0, N, 2048):
                    xs = nsb.tile([C, 2048], F32, tag="xs")
                    nc.scalar.activation(xs[:], xc[:, c0 : c0 + 2048], AF.Square)
                    ps = nps.tile([1, 2048], F32, tag="red")
                    for j in range(0, 2048, 512):
                        nc.tensor.matmul(
                            ps[:, j : j + 512], ones64[:], xs[:, j : j + 512]
                        )
                    sr = nsb.tile([1, 2048], F32, tag="sr")
                    nc.scalar.activation(sr[:], ps[:], AF.Sqrt)
                    nc.vector.reciprocal(rinv[:, c0 : c0 + 2048], sr[:])
                # broadcast rinv to 64 partitions by doubling DMAs
                nc.sync.dma_start(nb[0:1, :], rinv[:])
                for k in [1, 2, 4, 8, 16, 32]:
                    nc.sync.dma_start(nb[k : 2 * k, :], nb[0:k, :])
                nc.vector.tensor_tensor(xn[:], xc[:], nb[:], op=ALU.mult)
                # hi/lo split
                nc.scalar.activation(sq_c[0:C, :], xn[:], AF.Copy)
                nc.vector.tensor_tensor(
                    sq_c[C:128, :], xn[:], sq_c[0:C, :], op=ALU.subtract
                )
                nc.sync.dma_start(rx_c[C:128, :], sq_c[0:C, :])
                nc.sync.dma_start(rx_c[0:C, :], sq_c[C:128, :])

                # ---- queries: raw bf16 split (no normalize needed) ----
                xq = nsb.tile([C, Q], F32, tag="xq", bufs=1)
                nc.sync.dma_start(xq[:], xq_d[:])
                nc.scalar.activation(sq_q[0:C, :], xq[:], AF.Copy)
                nc.vector.tensor_tensor(
                    sq_q[C:128, :], xq[:], sq_q[0:C, :], op=ALU.subtract
                )

            with (
                tc.tile_pool(name="ndp", bufs=2) as ndp,
                tc.tile_pool(name="mps", bufs=2, space="PSUM") as mps,
                tc.tile_pool(name="smp", bufs=3) as smp,
            ):

                def main_phase():
                    main_body(nc, ndp, mps, smp, sq_q, sq_c, rx_c, out_d)

                if loop_iters > 1:
                    with tc.For_i(0, loop_iters, 1):
                        main_phase()
                else:
                    main_phase()
    return nc


def main_body(nc, ndp, mps, smp, sq_q, sq_c, rx_c, out_d):
    for qt in range(Q // QTS):
        q0 = qt * QTS
        negd = ndp.tile([QTS, N], F32, tag="negd")
        lhs_hi = sq_q[0:C, q0 : q0 + QTS]
        lhs_x = sq_q[:, q0 : q0 + QTS]
        for c0 in range(0, N, 2048):
            ps = mps.tile([QTS, 2048], F32, tag="mm")
            for j in range(0, 2048, 512):
                s = slice(c0 + j, c0 + j + 512)
                nc.tensor.matmul(
                    ps[:, j : j + 512], lhs_hi, sq_c[0:C, s],
                    start=True, stop=False,
                )
                nc.tensor.matmul(
                    ps[:, j : j + 512], lhs_x, rx_c[:, s],
                    start=False, stop=True,
                )
            nc.scalar.copy(negd[:, c0 : c0 + 2048], ps[:])
        cp = smp.tile([QTS, CK], F32, tag="cp")
        for g in range(NG):
            nc.vector.max(
                cp[:, g * 8 : (g + 1) * 8],
                negd[:, g * GRP : (g + 1) * GRP],
            )
        m = smp.tile([QTS, 24], F32, tag="m")
        cp2 = smp.tile([QTS, CK], F32, tag="cp2")
        cp3 = smp.tile([QTS, CK], F32, tag="cp3")
        nc.vector.max(m[:, 0:8], cp[:])
        nc.vector.match_replace(cp2[:], m[:, 0:8], cp[:], NEG_INF)
        nc.vector.max(m[:, 8:16], cp2[:])
        nc.vector.match_replace(cp3[:], m[:, 8:16], cp2[:], NEG_INF)
        nc.vector.max(m[:, 16:24], cp3[:])
        oidx = smp.tile([QTS, 8], U32, tag="oi")
        nc.vector.max_index(oidx[:], m[:, 2:17:2], negd[:])
        nc.sync.dma_start(out_d[q0 : q0 + QTS, :], oidx[:])


def kernel(x: np.ndarray) -> np.ndarray:
    x = np.asarray(x, dtype=np.float32)
    assert x.shape == (B, C, N, 1), x.shape
    xsq = x[..., 0]  # (B, C, N)

    nc = build_program()
    nc.finalize()

    in_maps = []
    for core in range(8):
        b, qi = divmod(core, 4)
        q0 = qi * Q
        in_maps.append(
            {
                "xb": np.ascontiguousarray(xsq[b]),
                "xq": np.ascontiguousarray(xsq[b][:, q0 : q0 + Q]),
            }
        )
    t0 = time.perf_counter_ns()
    res = run_bass_kernel_spmd(nc, in_maps, list(range(8)))
    t1 = time.perf_counter_ns()
    global _last_run
    _last_run = {
        "exec_time_ns": res.exec_time_ns,
        "mean_exec_time_ns": res.mean_exec_time_ns,
        "wall_ns": t1 - t0,
    }

    nn = np.empty((B, N, 9), dtype=np.int32)
    ar = np.arange(N, dtype=np.int32)
    nn[:, :, 0] = ar[None, :]
    for core in range(8):
        b, qi = divmod(core, 4)
        q0 = qi * Q
        nn[b, q0 : q0 + Q, 1:9] = res.results[core]["out"].astype(np.int32)
    center = np.broadcast_to(ar[None, :, None], (B, N, 9))
    return np.stack((nn, center), axis=0)


if __name__ == "__main__":
    rng = np.random.default_rng(0)
    x = rng.standard_normal((B, C, N, 1), dtype=np.float32)
    out = kernel(x=x)
    print(out.shape, out.dtype)
    print(out[0, 0, :3])


# revision 12
# speedup vs baseline: 27817.6315x; 27817.6315x over previous
"""DenseDilatedKnnGraph kernel for 8 Trainium2 NeuronCores.

Input : x (2, 64, 8192, 1) float32
Output: edge_index (2, 2, 8192, 9) int32
  out[0] = nn_idx[..., ::2] of top-18 nearest (L2, channel-normalized points)
  out[1] = center indices (arange broadcast)

Sharding: data-parallel over (batch, query-block): core c handles batch c//4,
queries [(c%4)*2048, (c%4+1)*2048). Each core holds all 8192 candidates.

Per-core pipeline (all ranking done on similarity = <q_raw, c_normalized>;
query-side normalization is a per-row scale and hence order-neutral):
  - normalize candidates (Square -> matmul-reduce -> Rsqrt -> mult)
  - split candidates and raw queries into bf16 hi/lo pairs; the distance
    matmul is hi.hi + (hi.lo + lo.hi) via two accumulating bf16 matmuls
    (~1e-5 abs accuracy, 4x faster than f32 matmul, p-state tolerant)
  - Act copies PSUM chunks to SBUF f32 (negd, double buffered)
  - DVE: 8x max8 over 1024-wide groups -> compact top-8-per-group list (64)
    -> 3 rounds max8/match_replace -> ranks 0..23 -> strided even ranks
    2,4,...,16 -> one full-row max_index for original indices
Rank 0 of the top-18 is always the query itself, filled host-side.
"""

import os
import sys
import time

import numpy as np

try:
    import concourse.bass as bass  # noqa: F401
except ImportError:  # fresh grading dir: make repo importable
    sys.path.append("/opt/trn_rl_repo")

import concourse.bacc as bacc
import concourse.mybir as mybir
import concourse.tile as tile
from concourse.bass_utils import run_bass_kernel_spmd

F32 = mybir.dt.float32
BF16 = mybir.dt.bfloat16
U32 = mybir.dt.uint32
AF = mybir.ActivationFunctionType
ALU = mybir.AluOpType

B = 2          # batch
C = 64         # channels
N = 8192       # points (candidates per core)
Q = 2048       # queries per core
QTS = 128      # queries per tile
GRP = 1024     # first-level group size
NG = N // GRP  # 8 groups
CK = NG * 8    # compact entries per row (64)
NEG_INF = -3.0e38


def build_program(loop_iters: int = 1):
    nc = bacc.Bacc()
    xb_d = nc.dram_tensor("xb", [C, N], F32, kind="ExternalInput")
    xq_d = nc.dram_tensor("xq", [C, Q], F32, kind="ExternalInput")
    out_d = nc.dram_tensor("out", [Q, 8], U32, kind="ExternalOutput")

    with tile.TileContext(nc) as tc:
        with (
            tc.tile_pool(name="const", bufs=1) as cst,
            tc.tile_pool(name="big", bufs=1) as big,
        ):
            ones64 = cst.tile([C, 1], F32)
            nc.gpsimd.memset(ones64[:], 1.0)

            # persistent bf16 operands
            sq_c = big.tile([128, N], BF16)   # candidates: [hi; lo]
            rx_c = big.tile([128, N], BF16)   # candidates: [lo; hi]
            sq_q = big.tile([128, Q], BF16)   # queries:    [hi; lo]
            lo_q = big.tile([C, Q], BF16)     # queries: lo at base partition 0

            with (
                tc.tile_pool(name="nsb", bufs=2) as nsb,
                tc.tile_pool(name="nps", bufs=2, space="PSUM") as nps,
            ):
                # ---- candidates: load, normalize, bf16 split ----
                xc = nsb.tile([C, N], F32, tag="xc", bufs=1)
                nc.sync.dma_start(xc[:], xb_d[:])
                xn = nsb.tile([C, N], F32, tag="xn", bufs=1)
                for c0 in range(0, N, 2048):
                    xs = nsb.tile([C, 2048], F32, tag="xs")
                    nc.scalar.activation(xs[:], xc[:, c0 : c0 + 2048], AF.Square)
                    ps = nps.tile([1, 2048], F32, tag="red")
                    for j in range(0, 2048, 512):
                        nc.tensor.matmul(
                            ps[:, j : j + 512], ones64[:], xs[:, j : j + 512]
                        )
                    sr = nsb.tile([1, 2048], F32, tag="sr")
                    nc.scalar.activation(sr[:], ps[:], AF.Sqrt)
                    # broadcast 1/norm to 64 partitions by doubling DMAs
                    nb = nsb.tile([C, 2048], F32, tag="nb")
                    nc.vector.reciprocal(nb[0:1, :], sr[:])
                    for k in [1, 2, 4, 8, 16, 32]:
                        nc.sync.dma_start(nb[k : 2 * k, :], nb[0:k, :])
                    nc.vector.tensor_tensor(
                        xn[:, c0 : c0 + 2048], xc[:, c0 : c0 + 2048], nb[:],
                        op=ALU.mult,
                    )
                # hi/lo split
                nc.scalar.activation(sq_c[0:C, :], xn[:], AF.Copy)
                nc.vector.tensor_tensor(
                    sq_c[C:128, :], xn[:], sq_c[0:C, :], op=ALU.subtract
                )
                nc.sync.dma_start(rx_c[C:128, :], sq_c[0:C, :])
                nc.sync.dma_start(rx_c[0:C, :], sq_c[C:128, :])

                # ---- queries: raw bf16 split (no normalize needed) ----
                xq = nsb.tile([C, Q], F32, tag="xq", bufs=1)
                nc.sync.dma_start(xq[:], xq_d[:])
                nc.scalar.activation(sq_q[0:C, :], xq[:], AF.Copy)
                nc.vector.tensor_tensor(
                    sq_q[C:128, :], xq[:], sq_q[0:C, :], op=ALU.subtract
                )
                nc.sync.dma_start(lo_q[:], sq_q[C:128, :])

            with (
                tc.tile_pool(name="ndp", bufs=2) as ndp,
                tc.tile_pool(name="mps", bufs=2, space="PSUM") as mps,
                tc.tile_pool(name="smp", bufs=3) as smp,
            ):

                def main_phase():
                    main_body(nc, ndp, mps, smp, sq_q, lo_q, sq_c, rx_c, out_d)

                if loop_iters > 1:
                    with tc.For_i(0, loop_iters, 1):
                        main_phase()
                else:
                    main_phase()
    return nc


def main_body(nc, ndp, mps, smp, sq_q, lo_q, sq_c, rx_c, out_d):
    for qt in range(Q // QTS):
        q0 = qt * QTS
        negd = ndp.tile([QTS, N], F32, tag="negd")
        lhs_hi = sq_q[0:C, q0 : q0 + QTS]
        lhs_x = sq_q[:, q0 : q0 + QTS]
        lhs_lo = lo_q[:, q0 : q0 + QTS]
        for c0 in range(0, N, 2048):
            ps = mps.tile([QTS, 2048], F32, tag="mm")
            for j in range(0, 2048, 512):
                s = slice(c0 + j, c0 + j + 512)
                nc.tensor.matmul(
                    ps[:, j : j + 512], lhs_hi, sq_c[0:C, s],
                    start=True, stop=False,
                )
                nc.tensor.matmul(
                    ps[:, j : j + 512], lhs_x, rx_c[:, s],
                    start=False, stop=False,
                )
                nc.tensor.matmul(
                    ps[:, j : j + 512], lhs_lo, rx_c[0:C, s],
                    start=False, stop=True,
                )
            nc.scalar.copy(negd[:, c0 : c0 + 2048], ps[:])
        cp = smp.tile([QTS, CK], F32, tag="cp")
        for g in range(NG):
            nc.vector.max(
                cp[:, g * 8 : (g + 1) * 8],
                negd[:, g * GRP : (g + 1) * GRP],
            )
        m = smp.tile([QTS, 24], F32, tag="m")
        cp2 = smp.tile([QTS, CK], F32, tag="cp2")
        cp3 = smp.tile([QTS, CK], F32, tag="cp3")
        nc.vector.max(m[:, 0:8], cp[:])
        nc.vector.match_replace(cp2[:], m[:, 0:8], cp[:], NEG_INF)
        nc.vector.max(m[:, 8:16], cp2[:])
        nc.vector.match_replace(cp3[:], m[:, 8:16], cp2[:], NEG_INF)
        nc.vector.max(m[:, 16:24], cp3[:])
        oidx = smp.tile([QTS, 8], U32, tag="oi")
        nc.vector.max_index(oidx[:], m[:, 2:17:2], negd[:])
        nc.sync.dma_start(out_d[q0 : q0 + QTS, :], oidx[:])


def kernel(x: np.ndarray) -> np.ndarray:
    x = np.asarray(x, dtype=np.float32)
    assert x.shape == (B, C, N, 1), x.shape
    xsq = x[..., 0]  # (B, C, N)

    nc = build_program()
    nc.finalize()

    in_maps = []
    for core in range(8):
        b, qi = divmod(core, 4)
        q0 = qi * Q
        in_maps.append(
            {
                "xb": np.ascontiguousarray(xsq[b]),
                "xq": np.ascontiguousarray(xsq[b][:, q0 : q0 + Q]),
            }
        )
    t0 = time.perf_counter_ns()
    res = run_bass_kernel_spmd(nc, in_maps, list(range(8)))
    t1 = time.perf_counter_ns()
    global _last_run
    _last_run = {
        "exec_time_ns": res.exec_time_ns,
        "mean_exec_time_ns": res.mean_exec_time_ns,
        "wall_ns": t1 - t0,
    }

    nn = np.empty((B, N, 9), dtype=np.int32)
    ar = np.arange(N, dtype=np.int32)
    nn[:, :, 0] = ar[None, :]
    for core in range(8):
        b, qi = divmod(core, 4)
        q0 = qi * Q
        nn[b, q0 : q0 + Q, 1:9] = res.results[core]["out"].astype(np.int32)
    center = np.broadcast_to(ar[None, :, None], (B, N, 9))
    return np.stack((nn, center), axis=0)


if __name__ == "__main__":
    rng = np.random.default_rng(0)
    x = rng.standard_normal((B, C, N, 1), dtype=np.float32)
    out = kernel(x=x)
    print(out.shape, out.dtype)
    print(out[0, 0, :3])


# revision 24
# speedup vs baseline: 189295.6476x; 6.8049x over previous
"""DenseDilatedKnnGraph kernel for 8 Trainium2 NeuronCores.

Input : x (2, 64, 8192, 1) float32
Output: edge_index (2, 2, 8192, 9) int32
  out[0] = nn_idx[..., ::2] of top-18 nearest (L2, channel-normalized points)
  out[1] = center indices (arange broadcast)

Sharding: data-parallel over (batch, query-block): core c handles batch c//4,
queries [(c%4)*2048, (c%4+1)*2048). Each core holds all 8192 candidates.

Per-core pipeline (all ranking done on similarity = <q_raw, c_normalized>;
query-side normalization is a per-row scale and hence order-neutral):
  - normalize candidates (Square -> matmul-reduce -> Rsqrt -> mult)
  - split candidates and raw queries into bf16 hi/lo pairs; the distance
    matmul is hi.hi + (hi.lo + lo.hi) via two accumulating bf16 matmuls
    (~1e-5 abs accuracy, 4x faster than f32 matmul, p-state tolerant)
  - Act copies PSUM chunks to SBUF f32 (negd, double buffered)
  - DVE: 8x max8 over 1024-wide groups -> compact top-8-per-group list (64)
    -> 3 rounds max8/match_replace -> ranks 0..23 -> strided even ranks
    2,4,...,16 -> one full-row max_index for original indices
Rank 0 of the top-18 is always the query itself, filled host-side.
"""

import os
import sys
import time

import numpy as np

try:
    import concourse.bass as bass  # noqa: F401
except ImportError:  # fresh grading dir: make repo importable
    sys.path.append("/opt/trn_rl_repo")

import concourse.bacc as bacc
import concourse.mybir as mybir
import concourse.tile as tile
from concourse.bass_utils import run_bass_kernel_spmd

F32 = mybir.dt.float32
BF16 = mybir.dt.bfloat16
U32 = mybir.dt.uint32
AF = mybir.ActivationFunctionType
ALU = mybir.AluOpType

B = 2          # batch
C = 64         # channels
N = 8192       # points (candidates per core)
Q = 2048       # queries per core
QTS = 128      # queries per tile
GRP = 1024     # first-level group size
NG = N // GRP  # 8 groups
CK = NG * 8    # compact entries per row (64)
NEG_INF = -3.0e38


def build_program(loop_iters: int = 1, parts: str = "full"):
    nc = bacc.Bacc()
    xb_d = nc.dram_tensor("xb", [C, N], F32, kind="ExternalInput")
    xq_d = nc.dram_tensor("xq", [C, Q], F32, kind="ExternalInput")
    out_d = nc.dram_tensor("out", [Q, 8], U32, kind="ExternalOutput")

    with tile.TileContext(nc) as tc:
        with (
            tc.tile_pool(name="const", bufs=1) as cst,
            tc.tile_pool(name="big", bufs=1) as big,
        ):
            ones64 = cst.tile([C, 1], F32)
            nc.gpsimd.memset(ones64[:], 1.0)

            # persistent bf16 operands
            sq_c = big.tile([128, N], BF16)   # candidates: [hi; lo]
            rx_c = big.tile([128, N], BF16)   # candidates: [lo; hi]
            sq_q = big.tile([128, Q], BF16)   # queries:    [hi; lo]

            with (
                tc.tile_pool(name="nsb", bufs=2) as nsb,
                tc.tile_pool(name="nps", bufs=2, space="PSUM") as nps,
            ):
                # ---- candidates: load, normalize, bf16 split ----
                xc = nsb.tile([C, N], F32, tag="xc", bufs=1)
                nc.sync.dma_start(xc[:], xb_d[:])
                xn = nsb.tile([C, N], F32, tag="xn", bufs=1)
                for c0 in range(0, N, 2048):
                    xs = nsb.tile([C, 2048], F32, tag="xs")
                    nc.scalar.activation(xs[:], xc[:, c0 : c0 + 2048], AF.Square)
                    ps = nps.tile([1, 2048], F32, tag="red")
                    for j in range(0, 2048, 512):
                        nc.tensor.matmul(
                            ps[:, j : j + 512], ones64[:], xs[:, j : j + 512]
                        )
                    sr = nsb.tile([1, 2048], F32, tag="sr")
                    nc.scalar.activation(sr[:], ps[:], AF.Sqrt)
                    # broadcast 1/norm to 64 partitions by doubling DMAs
                    nb = nsb.tile([C, 2048], F32, tag="nb")
                    nc.vector.reciprocal(nb[0:1, :], sr[:])
                    for k in [1, 2, 4, 8, 16, 32]:
                        nc.sync.dma_start(nb[k : 2 * k, :], nb[0:k, :])
                    nc.vector.tensor_tensor(
                        xn[:, c0 : c0 + 2048], xc[:, c0 : c0 + 2048], nb[:],
                        op=ALU.mult,
                    )
                # hi/lo split
                nc.scalar.activation(sq_c[0:C, :], xn[:], AF.Copy)
                nc.vector.tensor_tensor(
                    sq_c[C:128, :], xn[:], sq_c[0:C, :], op=ALU.subtract
                )
                nc.sync.dma_start(rx_c[C:128, :], sq_c[0:C, :])
                nc.sync.dma_start(rx_c[0:C, :], sq_c[C:128, :])

                # ---- queries: raw bf16 split (no normalize needed) ----
                xq = nsb.tile([C, Q], F32, tag="xq", bufs=1)
                nc.sync.dma_start(xq[:], xq_d[:])
                nc.scalar.activation(sq_q[0:C, :], xq[:], AF.Copy)
                nc.vector.tensor_tensor(
                    sq_q[C:128, :], xq[:], sq_q[0:C, :], op=ALU.subtract
                )


            with (
                tc.tile_pool(name="ndp", bufs=2) as ndp,
                tc.tile_pool(name="mps", bufs=2, space="PSUM") as mps,
                tc.tile_pool(name="smp", bufs=3) as smp,
            ):

                def main_phase():
                    main_body(nc, ndp, mps, smp, sq_q, sq_c, rx_c, out_d,
                              parts)

                if loop_iters > 1:
                    with tc.For_i(0, loop_iters, 1):
                        main_phase()
                else:
                    main_phase()
    return nc


def main_body(nc, ndp, mps, smp, sq_q, sq_c, rx_c, out_d, parts="full"):
    for qt in range(Q // QTS):
        q0 = qt * QTS
        negd = ndp.tile([QTS, N], F32, tag="negd")
        lhs_hi = sq_q[0:C, q0 : q0 + QTS]
        lhs_x = sq_q[:, q0 : q0 + QTS]
        cp = smp.tile([QTS, CK], F32, tag="cp")
        for c0 in range(0, N, 2048):
            ps = mps.tile([QTS, 2048], F32, tag="mm")
            # group by term so the PE loads weights twice per chunk, not 8x;
            # [hi;lo].T @ [hi;lo] + [hi;lo].T @ [lo;hi] = full (hi+lo) product
            for j in range(0, 2048, 512):
                s = slice(c0 + j, c0 + j + 512)
                nc.tensor.matmul(
                    ps[:, j : j + 512], lhs_x, sq_c[:, s],
                    start=True, stop=False,
                )
            for j in range(0, 2048, 512):
                s = slice(c0 + j, c0 + j + 512)
                nc.tensor.matmul(
                    ps[:, j : j + 512], lhs_x, rx_c[:, s],
                    start=False, stop=True,
                )
            # first-level top-8 per 1024-group straight from PSUM (keeps the
            # DVE stream independent of the Act copy, which only feeds the
            # final max_index)
            for g2 in range(2):
                g = c0 // GRP + g2
                nc.vector.max(
                    cp[:, g * 8 : (g + 1) * 8],
                    ps[:, g2 * GRP : (g2 + 1) * GRP],
                )
            nc.scalar.copy(negd[:, c0 : c0 + 2048], ps[:])
        if parts == "mm":
            nc.sync.dma_start(
                out_d[q0 : q0 + QTS, :], negd[:, 0:8].bitcast(U32)
            )
            continue
        m = smp.tile([QTS, 24], F32, tag="m")
        cp2 = smp.tile([QTS, CK], F32, tag="cp2")
        cp3 = smp.tile([QTS, CK], F32, tag="cp3")
        nc.vector.max(m[:, 0:8], cp[:])
        nc.vector.match_replace(cp2[:], m[:, 0:8], cp[:], NEG_INF)
        nc.vector.max(m[:, 8:16], cp2[:])
        nc.vector.match_replace(cp3[:], m[:, 8:16], cp2[:], NEG_INF)
        nc.vector.max(m[:, 16:24], cp3[:])
        if parts == "merge":
            nc.sync.dma_start(
                out_d[q0 : q0 + QTS, :], m[:, 0:8].bitcast(U32)
            )
            continue
        oidx = smp.tile([QTS, 8], U32, tag="oi")
        nc.vector.max_index(oidx[:], m[:, 2:17:2], negd[:])
        nc.sync.dma_start(out_d[q0 : q0 + QTS, :], oidx[:])


def kernel(x: np.ndarray) -> np.ndarray:
    x = np.asarray(x, dtype=np.float32)
    assert x.shape == (B, C, N, 1), x.shape
    xsq = x[..., 0]  # (B, C, N)

    nc = build_program()
    nc.finalize()

    in_maps = []
    for core in range(8):
        b, qi = divmod(core, 4)
        q0 = qi * Q
        in_maps.append(
            {
                "xb": np.ascontiguousarray(xsq[b]),
                "xq": np.ascontiguousarray(xsq[b][:, q0 : q0 + Q]),
            }
        )
    t0 = time.perf_counter_ns()
    res = run_bass_kernel_spmd(nc, in_maps, list(range(8)))
    t1 = time.perf_counter_ns()
    global _last_run
    _last_run = {
        "exec_time_ns": res.exec_time_ns,
        "mean_exec_time_ns": res.mean_exec_time_ns,
        "wall_ns": t1 - t0,
    }

    nn = np.empty((B, N, 9), dtype=np.int32)
    ar = np.arange(N, dtype=np.int32)
    nn[:, :, 0] = ar[None, :]
    for core in range(8):
        b, qi = divmod(core, 4)
        q0 = qi * Q
        nn[b, q0 : q0 + Q, 1:9] = res.results[core]["out"].astype(np.int32)
    center = np.broadcast_to(ar[None, :, None], (B, N, 9))
    return np.stack((nn, center), axis=0)


if __name__ == "__main__":
    rng = np.random.default_rng(0)
    x = rng.standard_normal((B, C, N, 1), dtype=np.float32)
    out = kernel(x=x)
    print(out.shape, out.dtype)
    print(out[0, 0, :3])


# revision 26
# speedup vs baseline: 229228.9035x; 1.2110x over previous
"""DenseDilatedKnnGraph kernel for 8 Trainium2 NeuronCores.

Input : x (2, 64, 8192, 1) float32
Output: edge_index (2, 2, 8192, 9) int32
  out[0] = nn_idx[..., ::2] of top-18 nearest (L2, channel-normalized points)
  out[1] = center indices (arange broadcast)

Sharding: data-parallel over (batch, query-block): core c handles batch c//4,
queries [(c%4)*2048, (c%4+1)*2048). Each core holds all 8192 candidates.

Per-core pipeline (all ranking done on similarity = <q_raw, c_normalized>;
query-side normalization is a per-row scale and hence order-neutral):
  - normalize candidates (Square -> matmul-reduce -> Rsqrt -> mult)
  - split candidates and raw queries into bf16 hi/lo pairs; the distance
    matmul is hi.hi + (hi.lo + lo.hi) via two accumulating bf16 matmuls
    (~1e-5 abs accuracy, 4x faster than f32 matmul, p-state tolerant)
  - Act copies PSUM chunks to SBUF f32 (negd, double buffered)
  - DVE: 8x max8 over 1024-wide groups -> compact top-8-per-group list (64)
    -> 3 rounds max8/match_replace -> ranks 0..23 -> strided even ranks
    2,4,...,16 -> one full-row max_index for original indices
Rank 0 of the top-18 is always the query itself, filled host-side.
"""

import os
import sys
import time

import numpy as np

try:
    import concourse.bass as bass  # noqa: F401
except ImportError:  # fresh grading dir: make repo importable
    sys.path.append("/opt/trn_rl_repo")

import concourse.bacc as bacc
import concourse.mybir as mybir
import concourse.tile as tile
from concourse.bass_utils import run_bass_kernel_spmd

F32 = mybir.dt.float32
BF16 = mybir.dt.bfloat16
U32 = mybir.dt.uint32
AF = mybir.ActivationFunctionType
ALU = mybir.AluOpType

B = 2          # batch
C = 64         # channels
N = 8192       # points (candidates per core)
Q = 2048       # queries per core
QTS = 128      # queries per tile
GRP = 1024     # first-level group size
NG = N // GRP  # 8 groups
CK = NG * 8    # compact entries per row (64)
NEG_INF = -3.0e38


def build_program(loop_iters: int = 1, parts: str = "full"):
    nc = bacc.Bacc()
    xb_d = nc.dram_tensor("xb", [C, N], F32, kind="ExternalInput")
    xq_d = nc.dram_tensor("xq", [C, Q], F32, kind="ExternalInput")
    out_d = nc.dram_tensor("out", [Q, 8], U32, kind="ExternalOutput")

    with tile.TileContext(nc) as tc:
        with (
            tc.tile_pool(name="const", bufs=1) as cst,
            tc.tile_pool(name="big", bufs=1) as big,
        ):
            ones64 = cst.tile([C, 1], F32)
            nc.gpsimd.memset(ones64[:], 1.0)

            # persistent bf16 operands
            sq_c = big.tile([128, N], BF16)   # candidates: [hi; lo]
            rx_c = big.tile([128, N], BF16)   # candidates: [lo; hi]
            sq_q = big.tile([128, Q], BF16)   # queries:    [hi; lo]

            with (
                tc.tile_pool(name="nsb", bufs=2) as nsb,
                tc.tile_pool(name="nps", bufs=2, space="PSUM") as nps,
            ):
                # ---- candidates: load, normalize, bf16 split ----
                xc = nsb.tile([C, N], F32, tag="xc", bufs=1)
                nc.sync.dma_start(xc[:], xb_d[:])
                xn = nsb.tile([C, N], F32, tag="xn", bufs=1)
                for c0 in range(0, N, 2048):
                    xs = nsb.tile([C, 2048], F32, tag="xs")
                    nc.scalar.activation(xs[:], xc[:, c0 : c0 + 2048], AF.Square)
                    ps = nps.tile([1, 2048], F32, tag="red")
                    for j in range(0, 2048, 512):
                        nc.tensor.matmul(
                            ps[:, j : j + 512], ones64[:], xs[:, j : j + 512]
                        )
                    sr = nsb.tile([1, 2048], F32, tag="sr")
                    nc.scalar.activation(sr[:], ps[:], AF.Sqrt)
                    # broadcast 1/norm to 64 partitions by doubling DMAs
                    nb = nsb.tile([C, 2048], F32, tag="nb")
                    nc.vector.reciprocal(nb[0:1, :], sr[:])
                    for k in [1, 2, 4, 8, 16, 32]:
                        nc.sync.dma_start(nb[k : 2 * k, :], nb[0:k, :])
                    nc.vector.tensor_tensor(
                        xn[:, c0 : c0 + 2048], xc[:, c0 : c0 + 2048], nb[:],
                        op=ALU.mult,
                    )
                # hi/lo split
                nc.scalar.activation(sq_c[0:C, :], xn[:], AF.Copy)
                nc.vector.tensor_tensor(
                    sq_c[C:128, :], xn[:], sq_c[0:C, :], op=ALU.subtract
                )
                nc.sync.dma_start(rx_c[C:128, :], sq_c[0:C, :])
                nc.sync.dma_start(rx_c[0:C, :], sq_c[C:128, :])

                # ---- queries: raw bf16 split (no normalize needed) ----
                xq = nsb.tile([C, Q], F32, tag="xq", bufs=1)
                nc.sync.dma_start(xq[:], xq_d[:])
                nc.scalar.activation(sq_q[0:C, :], xq[:], AF.Copy)
                nc.vector.tensor_tensor(
                    sq_q[C:128, :], xq[:], sq_q[0:C, :], op=ALU.subtract
                )


            with (
                tc.tile_pool(name="ndp", bufs=3) as ndp,
                tc.tile_pool(name="mps", bufs=2, space="PSUM") as mps,
                tc.tile_pool(name="smp", bufs=3) as smp,
            ):

                def main_phase():
                    main_body(nc, ndp, mps, smp, sq_q, sq_c, rx_c, out_d,
                              parts)

                if loop_iters > 1:
                    with tc.For_i(0, loop_iters, 1):
                        main_phase()
                else:
                    main_phase()
    return nc


def main_body(nc, ndp, mps, smp, sq_q, sq_c, rx_c, out_d, parts="full"):
    for qt in range(Q // QTS):
        q0 = qt * QTS
        negd = ndp.tile([QTS, N], F32, tag="negd")
        lhs_hi = sq_q[0:C, q0 : q0 + QTS]
        lhs_x = sq_q[:, q0 : q0 + QTS]
        cp = smp.tile([QTS, CK], F32, tag="cp")
        for c0 in range(0, N, 2048):
            ps = mps.tile([QTS, 2048], F32, tag="mm")
            # group by term so the PE loads weights twice per chunk, not 8x;
            # [hi;lo].T @ [hi;lo] + [hi;lo].T @ [lo;hi] = full (hi+lo) product
            for j in range(0, 2048, 512):
                s = slice(c0 + j, c0 + j + 512)
                nc.tensor.matmul(
                    ps[:, j : j + 512], lhs_x, sq_c[:, s],
                    start=True, stop=False,
                )
            for j in range(0, 2048, 512):
                s = slice(c0 + j, c0 + j + 512)
                nc.tensor.matmul(
                    ps[:, j : j + 512], lhs_x, rx_c[:, s],
                    start=False, stop=True,
                )
            nc.scalar.copy(negd[:, c0 : c0 + 2048], ps[:])
        if parts == "mm":
            nc.sync.dma_start(
                out_d[q0 : q0 + QTS, :], negd[:, 0:8].bitcast(U32)
            )
            continue
        for g in range(NG):
            nc.vector.max(
                cp[:, g * 8 : (g + 1) * 8],
                negd[:, g * GRP : (g + 1) * GRP],
            )
        m = smp.tile([QTS, 24], F32, tag="m")
        cp2 = smp.tile([QTS, CK], F32, tag="cp2")
        cp3 = smp.tile([QTS, CK], F32, tag="cp3")
        nc.vector.max(m[:, 0:8], cp[:])
        nc.vector.match_replace(cp2[:], m[:, 0:8], cp[:], NEG_INF)
        nc.vector.max(m[:, 8:16], cp2[:])
        nc.vector.match_replace(cp3[:], m[:, 8:16], cp2[:], NEG_INF)
        nc.vector.max(m[:, 16:24], cp3[:])
        if parts == "merge":
            nc.sync.dma_start(
                out_d[q0 : q0 + QTS, :], m[:, 0:8].bitcast(U32)
            )
            continue
        oidx = smp.tile([QTS, 8], U32, tag="oi")
        nc.vector.max_index(oidx[:], m[:, 2:17:2], negd[:])
        nc.sync.dma_start(out_d[q0 : q0 + QTS, :], oidx[:])


def kernel(x: np.ndarray) -> np.ndarray:
    x = np.asarray(x, dtype=np.float32)
    assert x.shape == (B, C, N, 1), x.shape
    xsq = x[..., 0]  # (B, C, N)

    nc = build_program()
    nc.finalize()

    in_maps = []
    for core in range(8):
        b, qi = divmod(core, 4)
        q0 = qi * Q
        in_maps.append(
            {
                "xb": np.ascontiguousarray(xsq[b]),
                "xq": np.ascontiguousarray(xsq[b][:, q0 : q0 + Q]),
            }
        )
    t0 = time.perf_counter_ns()
    res = run_bass_kernel_spmd(nc, in_maps, list(range(8)))
    t1 = time.perf_counter_ns()
    global _last_run
    _last_run = {
        "exec_time_ns": res.exec_time_ns,
        "mean_exec_time_ns": res.mean_exec_time_ns,
        "wall_ns": t1 - t0,
    }

    nn = np.empty((B, N, 9), dtype=np.int32)
    ar = np.arange(N, dtype=np.int32)
    nn[:, :, 0] = ar[None, :]
    for core in range(8):
        b, qi = divmod(core, 4)
        q0 = qi * Q
        nn[b, q0 : q0 + Q, 1:9] = res.results[core]["out"].astype(np.int32)
    center = np.broadcast_to(ar[None, :, None], (B, N, 9))
    return np.stack((nn, center), axis=0)


if __name__ == "__main__":
    rng = np.random.default_rng(0)
    x = rng.standard_normal((B, C, N, 1), dtype=np.float32)
    out = kernel(x=x)
    print(out.shape, out.dtype)
    print(out[0, 0, :3])


# revision 32
# speedup vs baseline: 277011.3644x; 1.2084x over previous
"""DenseDilatedKnnGraph kernel for 8 Trainium2 NeuronCores.

Input : x (2, 64, 8192, 1) float32
Output: edge_index (2, 2, 8192, 9) int32
  out[0] = nn_idx[..., ::2] of top-18 nearest (L2, channel-normalized points)
  out[1] = center indices (arange broadcast)

Sharding: data-parallel over (batch, query-block): core c handles batch c//4,
queries [(c%4)*2048, (c%4+1)*2048). Each core holds all 8192 candidates.

Per-core pipeline (all ranking done on similarity = <q_raw, c_normalized>;
query-side normalization is a per-row scale and hence order-neutral):
  - normalize candidates (Square -> matmul-reduce -> Rsqrt -> mult)
  - split candidates and raw queries into bf16 hi/lo pairs; the distance
    matmul is hi.hi + (hi.lo + lo.hi) via two accumulating bf16 matmuls
    (~1e-5 abs accuracy, 4x faster than f32 matmul, p-state tolerant)
  - Act copies PSUM chunks to SBUF f32 (negd, double buffered)
  - DVE: 8x max8 over 1024-wide groups -> compact top-8-per-group list (64)
    -> 3 rounds max8/match_replace -> ranks 0..23 -> strided even ranks
    2,4,...,16 -> one full-row max_index for original indices
Rank 0 of the top-18 is always the query itself, filled host-side.
"""

import os
import sys
import time

import numpy as np

try:
    import concourse.bass as bass  # noqa: F401
except ImportError:  # fresh grading dir: make repo importable
    sys.path.append("/opt/trn_rl_repo")

import concourse.bacc as bacc
import concourse.mybir as mybir
import concourse.tile as tile
from concourse.bass_utils import run_bass_kernel_spmd

F32 = mybir.dt.float32
BF16 = mybir.dt.bfloat16
U32 = mybir.dt.uint32
AF = mybir.ActivationFunctionType
ALU = mybir.AluOpType

B = 2          # batch
C = 64         # channels
N = 8192       # points (candidates per core)
Q = 2048       # queries per core
QTS = 128      # queries per tile
GRP = 1024     # first-level group size
NG = N // GRP  # 8 groups
CK = NG * 8    # compact entries per row (64)
NEG_INF = -3.0e38


def build_program(loop_iters: int = 1, parts: str = "full"):
    nc = bacc.Bacc()
    xb_d = nc.dram_tensor("xb", [C, N], F32, kind="ExternalInput")
    xq_d = nc.dram_tensor("xq", [C, Q], F32, kind="ExternalInput")
    out_d = nc.dram_tensor("out", [Q, 8], U32, kind="ExternalOutput")

    with tile.TileContext(nc) as tc:
        with (
            tc.tile_pool(name="const", bufs=1) as cst,
            tc.tile_pool(name="big", bufs=1) as big,
        ):
            ones64 = cst.tile([C, 1], F32)
            nc.gpsimd.memset(ones64[:], 1.0)

            # persistent bf16 operands
            sq_c = big.tile([128, N], BF16)   # candidates: [hi; lo]
            rx_c = big.tile([128, N], BF16)   # candidates: [lo; hi]
            sq_q = big.tile([128, Q], BF16)   # queries:    [hi; lo]

            with (
                tc.tile_pool(name="nsb", bufs=2) as nsb,
                tc.tile_pool(name="nps", bufs=2, space="PSUM") as nps,
            ):
                # ---- candidates: load, normalize, bf16 split ----
                xc = nsb.tile([C, N], F32, tag="xc", bufs=1)
                nc.sync.dma_start(xc[:], xb_d[:])
                xn = nsb.tile([C, N], F32, tag="xn", bufs=1)
                for c0 in range(0, N, 2048):
                    xs = nsb.tile([C, 2048], F32, tag="xs")
                    nc.scalar.activation(xs[:], xc[:, c0 : c0 + 2048], AF.Square)
                    ps = nps.tile([1, 2048], F32, tag="red")
                    for j in range(0, 2048, 512):
                        nc.tensor.matmul(
                            ps[:, j : j + 512], ones64[:], xs[:, j : j + 512]
                        )
                    sr = nsb.tile([1, 2048], F32, tag="sr")
                    nc.scalar.activation(sr[:], ps[:], AF.Sqrt)
                    # broadcast 1/norm to 64 partitions by doubling DMAs
                    nb = nsb.tile([C, 2048], F32, tag="nb")
                    nc.vector.reciprocal(nb[0:1, :], sr[:])
                    for k in [1, 2, 4, 8, 16, 32]:
                        nc.sync.dma_start(nb[k : 2 * k, :], nb[0:k, :])
                    nc.vector.tensor_tensor(
                        xn[:, c0 : c0 + 2048], xc[:, c0 : c0 + 2048], nb[:],
                        op=ALU.mult,
                    )
                # hi/lo split
                nc.scalar.activation(sq_c[0:C, :], xn[:], AF.Copy)
                nc.vector.tensor_tensor(
                    sq_c[C:128, :], xn[:], sq_c[0:C, :], op=ALU.subtract
                )
                nc.sync.dma_start(rx_c[C:128, :], sq_c[0:C, :])
                nc.sync.dma_start(rx_c[0:C, :], sq_c[C:128, :])

                # ---- queries: raw bf16 split (no normalize needed) ----
                xq = nsb.tile([C, Q], F32, tag="xq", bufs=1)
                nc.sync.dma_start(xq[:], xq_d[:])
                nc.scalar.activation(sq_q[0:C, :], xq[:], AF.Copy)
                nc.vector.tensor_tensor(
                    sq_q[C:128, :], xq[:], sq_q[0:C, :], op=ALU.subtract
                )


            with (
                tc.tile_pool(name="ndp", bufs=3) as ndp,
                tc.tile_pool(name="mps", bufs=4, space="PSUM") as mps,
                tc.tile_pool(name="smp", bufs=3) as smp,
            ):

                def main_phase():
                    main_body(nc, ndp, mps, smp, sq_q, sq_c, rx_c, out_d,
                              parts)

                if loop_iters > 1:
                    with tc.For_i(0, loop_iters, 1):
                        main_phase()
                else:
                    main_phase()
    return nc


def main_body(nc, ndp, mps, smp, sq_q, sq_c, rx_c, out_d, parts="full"):
    for qt in range(Q // QTS):
        q0 = qt * QTS
        negd = ndp.tile([QTS, N], F32, tag="negd")
        lhs_hi = sq_q[0:C, q0 : q0 + QTS]
        lhs_x = sq_q[:, q0 : q0 + QTS]
        cp = smp.tile([QTS, CK], F32, tag="cp")
        for c0 in range(0, N, 1024):
            psA = mps.tile([QTS, 512], F32, tag="mmA")
            psB = mps.tile([QTS, 512], F32, tag="mmB")
            sA = slice(c0, c0 + 512)
            sB = slice(c0 + 512, c0 + 1024)
            # [hi;lo].T @ [hi;lo] + [hi;lo].T @ [lo;hi] = full (hi+lo) product
            nc.tensor.matmul(psA[:], lhs_x, sq_c[:, sA], start=True, stop=False)
            nc.tensor.matmul(psB[:], lhs_x, sq_c[:, sB], start=True, stop=False)
            nc.tensor.matmul(psA[:], lhs_x, rx_c[:, sA], start=False, stop=True)
            nc.tensor.matmul(psB[:], lhs_x, rx_c[:, sB], start=False, stop=True)
            nc.scalar.copy(negd[:, sA], psA[:])
            nc.scalar.copy(negd[:, sB], psB[:])
        if parts == "mm":
            nc.sync.dma_start(
                out_d[q0 : q0 + QTS, :], negd[:, 0:8].bitcast(U32)
            )
            continue
        for g in range(NG):
            nc.vector.max(
                cp[:, g * 8 : (g + 1) * 8],
                negd[:, g * GRP : (g + 1) * GRP],
            )
        m = smp.tile([QTS, 24], F32, tag="m")
        cp2 = smp.tile([QTS, CK], F32, tag="cp2")
        cp3 = smp.tile([QTS, CK], F32, tag="cp3")
        nc.vector.max(m[:, 0:8], cp[:])
        nc.vector.match_replace(cp2[:], m[:, 0:8], cp[:], NEG_INF)
        nc.vector.max(m[:, 8:16], cp2[:])
        nc.vector.match_replace(cp3[:], m[:, 8:16], cp2[:], NEG_INF)
        nc.vector.max(m[:, 16:24], cp3[:])
        if parts == "merge":
            nc.sync.dma_start(
                out_d[q0 : q0 + QTS, :], m[:, 0:8].bitcast(U32)
            )
            continue
        oidx = smp.tile([QTS, 8], U32, tag="oi")
        nc.vector.max_index(oidx[:], m[:, 2:17:2], negd[:])
        nc.sync.dma_start(out_d[q0 : q0 + QTS, :], oidx[:])


def kernel(x: np.ndarray) -> np.ndarray:
    x = np.asarray(x, dtype=np.float32)
    assert x.shape == (B, C, N, 1), x.shape
    xsq = x[..., 0]  # (B, C, N)

    nc = build_program()
    nc.finalize()

    in_maps = []
    for core in range(8):
        b, qi = divmod(core, 4)
        q0 = qi * Q
        in_maps.append(
            {
                "xb": np.ascontiguousarray(xsq[b]),
                "xq": np.ascontiguousarray(xsq[b][:, q0 : q0 + Q]),
            }
        )
    t0 = time.perf_counter_ns()
    res = run_bass_kernel_spmd(nc, in_maps, list(range(8)))
    t1 = time.perf_counter_ns()
    global _last_run
    _last_run = {
        "exec_time_ns": res.exec_time_ns,
        "mean_exec_time_ns": res.mean_exec_time_ns,
        "wall_ns": t1 - t0,
    }

    nn = np.empty((B, N, 9), dtype=np.int32)
    ar = np.arange(N, dtype=np.int32)
    nn[:, :, 0] = ar[None, :]
    for core in range(8):
        b, qi = divmod(core, 4)
        q0 = qi * Q
        nn[b, q0 : q0 + Q, 1:9] = res.results[core]["out"].astype(np.int32)
    center = np.broadcast_to(ar[None, :, None], (B, N, 9))
    return np.stack((nn, center), axis=0)


if __name__ == "__main__":
    rng = np.random.default_rng(0)
    x = rng.standard_normal((B, C, N, 1), dtype=np.float32)
    out = kernel(x=x)
    print(out.shape, out.dtype)
    print(out[0, 0, :3])


# revision 38
# speedup vs baseline: 279013.7700x; 1.0072x over previous
"""DenseDilatedKnnGraph kernel for 8 Trainium2 NeuronCores.

Input : x (2, 64, 8192, 1) float32
Output: edge_index (2, 2, 8192, 9) int32
  out[0] = nn_idx[..., ::2] of top-18 nearest (L2, channel-normalized points)
  out[1] = center indices (arange broadcast)

Sharding: data-parallel over (batch, query-block): core c handles batch c//4,
queries [(c%4)*2048, (c%4+1)*2048). Each core holds all 8192 candidates.

Per-core pipeline (all ranking done on similarity = <q_raw, c_normalized>;
query-side normalization is a per-row scale and hence order-neutral):
  - normalize candidates (Square -> matmul-reduce -> sqrt -> reciprocal ->
    mult)
  - split candidates and raw queries into stacked bf16 hi/lo operands
    S=[hi;lo], T=[lo;hi]; the similarity matmul is S.T@S + S.T@T =
    the full (hi+lo).(hi'+lo') product via two accumulating K=128 bf16
    matmuls (~1e-6 accuracy, 4x fewer PE cycles than f32, p-state
    tolerant). PSUM tiles are 512 wide x 4 buffers per tag (all 8 banks),
    with matmuls paired over two tiles so weights reload only per term.
  - Act copies PSUM chunks to SBUF f32 (negd, triple buffered) - fine
    granularity keeps the PE->Act->DVE handoff bubble small (2048-wide
    chunks measured 65us/iter slower).
  - DVE: 8x max8 over 1024-wide groups -> compact top-8-per-group list (64)
    -> 3 rounds max8/match_replace -> ranks 0..23 -> strided even ranks
    2,4,...,16 -> one full-row max_index for original indices
Rank 0 of the top-18 is always the query itself, filled host-side.
Measured 312us/iter steady-state (baseline 463us); mm+copy stage alone
paces ~15us/qt, DVE ~17us/qt model; stages overlap to ~19.5us/qt.
"""

import os
import sys
import time

import numpy as np

try:
    import concourse.bass as bass  # noqa: F401
except ImportError:  # fresh grading dir: make repo importable
    sys.path.append("/opt/trn_rl_repo")

import concourse.bacc as bacc
import concourse.mybir as mybir
import concourse.tile as tile
from concourse.bass_utils import run_bass_kernel_spmd

F32 = mybir.dt.float32
BF16 = mybir.dt.bfloat16
U32 = mybir.dt.uint32
AF = mybir.ActivationFunctionType
ALU = mybir.AluOpType

B = 2          # batch
C = 64         # channels
N = 8192       # points (candidates per core)
Q = 2048       # queries per core
QTS = 128      # queries per tile
GRP = 1024     # first-level group size
NG = N // GRP  # 8 groups
CK = NG * 8    # compact entries per row (64)
NEG_INF = -3.0e38


def build_program(loop_iters: int = 1, parts: str = "full"):
    nc = bacc.Bacc()
    xb_d = nc.dram_tensor("xb", [C, N], F32, kind="ExternalInput")
    xq_d = nc.dram_tensor("xq", [C, Q], F32, kind="ExternalInput")
    out_d = nc.dram_tensor("out", [Q, 8], U32, kind="ExternalOutput")

    with tile.TileContext(nc) as tc:
        with (
            tc.tile_pool(name="const", bufs=1) as cst,
            tc.tile_pool(name="big", bufs=1) as big,
        ):
            ones64 = cst.tile([C, 1], F32)
            nc.gpsimd.memset(ones64[:], 1.0)

            # persistent bf16 operands
            sq_c = big.tile([128, N], BF16)   # candidates: [hi; lo]
            rx_c = big.tile([128, N], BF16)   # candidates: [lo; hi]
            sq_q = big.tile([128, Q], BF16)   # queries:    [hi; lo]

            with (
                tc.tile_pool(name="nsb", bufs=2) as nsb,
                tc.tile_pool(name="nps", bufs=2, space="PSUM") as nps,
            ):
                # ---- candidates: load, normalize, bf16 split ----
                xc = nsb.tile([C, N], F32, tag="xc", bufs=1)
                nc.sync.dma_start(xc[:], xb_d[:])
                xn = nsb.tile([C, N], F32, tag="xn", bufs=1)
                for c0 in range(0, N, 2048):
                    xs = nsb.tile([C, 2048], F32, tag="xs")
                    nc.scalar.activation(xs[:], xc[:, c0 : c0 + 2048], AF.Square)
                    ps = nps.tile([1, 2048], F32, tag="red")
                    for j in range(0, 2048, 512):
                        nc.tensor.matmul(
                            ps[:, j : j + 512], ones64[:], xs[:, j : j + 512]
                        )
                    sr = nsb.tile([1, 2048], F32, tag="sr")
                    nc.scalar.activation(sr[:], ps[:], AF.Sqrt)
                    # broadcast 1/norm to 64 partitions by doubling DMAs
                    nb = nsb.tile([C, 2048], F32, tag="nb")
                    nc.vector.reciprocal(nb[0:1, :], sr[:])
                    for k in [1, 2, 4, 8, 16, 32]:
                        nc.sync.dma_start(nb[k : 2 * k, :], nb[0:k, :])
                    nc.vector.tensor_tensor(
                        xn[:, c0 : c0 + 2048], xc[:, c0 : c0 + 2048], nb[:],
                        op=ALU.mult,
                    )
                # hi/lo split
                nc.scalar.activation(sq_c[0:C, :], xn[:], AF.Copy)
                nc.vector.tensor_tensor(
                    sq_c[C:128, :], xn[:], sq_c[0:C, :], op=ALU.subtract
                )
                nc.sync.dma_start(rx_c[C:128, :], sq_c[0:C, :])
                nc.sync.dma_start(rx_c[0:C, :], sq_c[C:128, :])

                # ---- queries: raw bf16 split (no normalize needed) ----
                xq = nsb.tile([C, Q], F32, tag="xq", bufs=1)
                nc.sync.dma_start(xq[:], xq_d[:])
                nc.scalar.activation(sq_q[0:C, :], xq[:], AF.Copy)
                nc.vector.tensor_tensor(
                    sq_q[C:128, :], xq[:], sq_q[0:C, :], op=ALU.subtract
                )


            with (
                tc.tile_pool(name="ndp", bufs=4) as ndp,
                tc.tile_pool(name="mps", bufs=4, space="PSUM") as mps,
                tc.tile_pool(name="smp", bufs=3) as smp,
            ):

                def main_phase():
                    main_body(nc, ndp, mps, smp, sq_q, sq_c, rx_c, out_d,
                              parts)

                if loop_iters > 1:
                    with tc.For_i(0, loop_iters, 1):
                        main_phase()
                else:
                    main_phase()
    return nc


def main_body(nc, ndp, mps, smp, sq_q, sq_c, rx_c, out_d, parts="full"):
    for qt in range(Q // QTS):
        q0 = qt * QTS
        negd = ndp.tile([QTS, N], F32, tag="negd")
        lhs_hi = sq_q[0:C, q0 : q0 + QTS]
        lhs_x = sq_q[:, q0 : q0 + QTS]
        cp = smp.tile([QTS, CK], F32, tag="cp")
        for c0 in range(0, N, 1024):
            psA = mps.tile([QTS, 512], F32, tag="mmA")
            psB = mps.tile([QTS, 512], F32, tag="mmB")
            sA = slice(c0, c0 + 512)
            sB = slice(c0 + 512, c0 + 1024)
            # [hi;lo].T @ [hi;lo] + [hi;lo].T @ [lo;hi] = full (hi+lo) product
            nc.tensor.matmul(psA[:], lhs_x, sq_c[:, sA], start=True, stop=False)
            nc.tensor.matmul(psB[:], lhs_x, sq_c[:, sB], start=True, stop=False)
            nc.tensor.matmul(psA[:], lhs_x, rx_c[:, sA], start=False, stop=True)
            nc.tensor.matmul(psB[:], lhs_x, rx_c[:, sB], start=False, stop=True)
            nc.scalar.copy(negd[:, sA], psA[:])
            nc.scalar.copy(negd[:, sB], psB[:])
        if parts == "mm":
            nc.sync.dma_start(
                out_d[q0 : q0 + QTS, :], negd[:, 0:8].bitcast(U32)
            )
            continue
        for g in range(NG):
            nc.vector.max(
                cp[:, g * 8 : (g + 1) * 8],
                negd[:, g * GRP : (g + 1) * GRP],
            )
        m = smp.tile([QTS, 24], F32, tag="m")
        cp2 = smp.tile([QTS, CK], F32, tag="cp2")
        cp3 = smp.tile([QTS, CK], F32, tag="cp3")
        nc.vector.max(m[:, 0:8], cp[:])
        nc.vector.match_replace(cp2[:], m[:, 0:8], cp[:], NEG_INF)
        nc.vector.max(m[:, 8:16], cp2[:])
        nc.vector.match_replace(cp3[:], m[:, 8:16], cp2[:], NEG_INF)
        nc.vector.max(m[:, 16:24], cp3[:])
        if parts == "merge":
            nc.sync.dma_start(
                out_d[q0 : q0 + QTS, :], m[:, 0:8].bitcast(U32)
            )
            continue
        oidx = smp.tile([QTS, 8], U32, tag="oi")
        nc.vector.max_index(oidx[:], m[:, 2:17:2], negd[:])
        nc.sync.dma_start(out_d[q0 : q0 + QTS, :], oidx[:])


def kernel(x: np.ndarray) -> np.ndarray:
    x = np.asarray(x, dtype=np.float32)
    assert x.shape == (B, C, N, 1), x.shape
    xsq = x[..., 0]  # (B, C, N)

    nc = build_program()
    nc.finalize()

    in_maps = []
    for core in range(8):
        b, qi = divmod(core, 4)
        q0 = qi * Q
        in_maps.append(
            {
                "xb": np.ascontiguousarray(xsq[b]),
                "xq": np.ascontiguousarray(xsq[b][:, q0 : q0 + Q]),
            }
        )
    t0 = time.perf_counter_ns()
    res = run_bass_kernel_spmd(nc, in_maps, list(range(8)))
    t1 = time.perf_counter_ns()
    global _last_run
    _last_run = {
        "exec_time_ns": res.exec_time_ns,
        "mean_exec_time_ns": res.mean_exec_time_ns,
        "wall_ns": t1 - t0,
    }

    nn = np.empty((B, N, 9), dtype=np.int32)
    ar = np.arange(N, dtype=np.int32)
    nn[:, :, 0] = ar[None, :]
    for core in range(8):
        b, qi = divmod(core, 4)
        q0 = qi * Q
        nn[b, q0 : q0 + Q, 1:9] = res.results[core]["out"].astype(np.int32)
    center = np.broadcast_to(ar[None, :, None], (B, N, 9))
    return np.stack((nn, center), axis=0)


if __name__ == "__main__":
    rng = np.random.default_rng(0)
    x = rng.standard_normal((B, C, N, 1), dtype=np.float32)
    out = kernel(x=x)
    print(out.shape, out.dtype)
    print(out[0, 0, :3])
